# revision 9
# baseline (speedup 1.0000x reference)
"""MoE router gate kernel for Trainium2 (8 NeuronCores, SPMD data-parallel).

Computes, for x [16384, 2048] f32 and W [8, 2048] f32:
    logits = x @ W.T                      [T, 8]
    probs  = softmax(logits, axis=1)
    vals, idx = top_k(probs, 2)
    weights = vals / vals.sum(-1, keepdims=True)
returns (weights [T,2] f32, idx [T,2] int32).

Key identity: the renormalized top-2 softmax weights only depend on the two
top logits: w0 = 1/(1+exp(l1-l0)), w1 = exp(l1-l0)/(1+exp(l1-l0)).

Layout trick: d=2048 is viewed as 128 partitions x 16 contiguous f32 (64B
chunks, DMA-efficient), so the PE can contract over d without transposing x.
16 matmuls (one per j in 0..15) accumulate in PSUM:
    logits[t, e] = sum_j sum_p x[t, 16p+j] * W[e, 16p+j]
Top-2 via the DVE Max/MaxIndex (top-8 sorted) instructions.
"""

import numpy as np

T_FULL = 16384
D = 2048
E = 8
N_CORES = 8
T_CORE = T_FULL // N_CORES  # 2048
P = 128
J = D // P  # 16 contiguous f32 per partition = 64B chunks
TOK_TILE = 128  # tokens per matmul/psum tile
DMA_TOK = 256  # tokens per input DMA tile

_cached = {}


def _build_bass():
    import concourse.bass as bass
    import concourse.mybir as mybir
    import concourse.tile as tile

    f32 = mybir.dt.float32
    u32 = mybir.dt.uint32

    nc = bass.Bass()
    x_in = nc.dram_tensor("x", [T_CORE, D], f32, kind="ExternalInput")
    w_in = nc.dram_tensor("w", [E, D], f32, kind="ExternalInput")
    out_w = nc.dram_tensor("out_w", [T_CORE, 2], f32, kind="ExternalOutput")
    out_i = nc.dram_tensor("out_i", [T_CORE, 2], u32, kind="ExternalOutput")

    # DRAM views:
    #   xr[n, p, tt, j] = x[n*DMA_TOK + tt, 16p + j]   (64B contiguous in j)
    xr = x_in.rearrange("(n tt) (p j) -> n p tt j", tt=DMA_TOK, p=P)
    wr = w_in.rearrange("e (p j) -> p e j", p=P)

    n_dma = T_CORE // DMA_TOK
    sub_per_dma = DMA_TOK // TOK_TILE

    with tile.TileContext(nc) as tc:
        with (
            tc.tile_pool(name="const", bufs=1) as cpool,
            tc.tile_pool(name="xg", bufs=8) as xpool,
            tc.tile_pool(name="psum", bufs=4, space="PSUM") as ppool,
            tc.tile_pool(name="dummy_psum", bufs=1, space="PSUM") as dpool,
        ):
            # The fp32 self-loading Matmult (S3_LW) only has room for a
            # single sync-wait command, but the first matmul touching a
            # freshly-DMA'd tile can need several (multi-queue DMA sems +
            # PSUM-bank reuse). Gate each DMA with a tiny 1x1 dummy matmul:
            # PE executes in order, so the dummy absorbs the DMA waits and
            # the real matmuls carry at most the PSUM-reuse wait.
            dps = dpool.tile([1, 1], f32)

            def pe_gate(ap):
                nc.tensor.matmul(dps[:], ap, ap, start=True, stop=True)

            wg = cpool.tile([P, E, J], f32)
            nc.sync.dma_start(wg[:], wr)
            pe_gate(wg[:, 0:1, 0])

            n_tiles = T_CORE // TOK_TILE
            # Every epilogue intermediate is a disjoint column range of a
            # persistent tile: no slot reuse -> no WAR/WAW deps -> each op
            # carries at most ONE sync wait (this walrus build rejects >1
            # wait per instruction). The l1-l0 subtract is folded into the
            # Exp's bias operand (an AP) so the ACT op only depends on DVE.
            w_stage = cpool.tile([P, n_tiles, 2], f32)
            i_stage = cpool.tile([P, n_tiles, 8], u32)
            logits_all = cpool.tile([P, n_tiles, E], f32)
            m8_all = cpool.tile([P, n_tiles, 8], f32)
            neg_all = cpool.tile([P, n_tiles], f32)
            ex_all = cpool.tile([P, n_tiles], f32)
            den_all = cpool.tile([P, n_tiles], f32)

            for n in range(n_dma):
                xg = xpool.tile([P, DMA_TOK, J], f32)
                nc.sync.dma_start(xg[:], xr[n])
                pe_gate(xg[:, 0:1, 0])

                for s in range(sub_per_dma):
                    t = n * sub_per_dma + s
                    ps = ppool.tile([TOK_TILE, E], f32)
                    for j in range(J):
                        nc.tensor.matmul(
                            ps[:],
                            xg[:, s * TOK_TILE : (s + 1) * TOK_TILE, j],
                            wg[:, :, j],
                            start=(j == 0),
                            stop=(j == J - 1),
                        )

                    logits = logits_all[:, t, :]
                    m8 = m8_all[:, t, :]
                    nc.vector.tensor_copy(logits, ps[:])
                    nc.vector.max(m8, logits)
                    nc.vector.max_index(i_stage[:, t, :], m8, logits)

                    # w0 = 1/(1+exp(l1-l0)); w1 = 1 - w0
                    nc.vector.tensor_scalar_mul(
                        neg_all[:, t : t + 1], m8[:, 0:1], -1.0
                    )
                    nc.scalar.activation(
                        ex_all[:, t : t + 1],
                        m8[:, 1:2],
                        mybir.ActivationFunctionType.Exp,
                        bias=neg_all[:, t : t + 1],
                    )
                    nc.vector.tensor_scalar_add(
                        den_all[:, t : t + 1], ex_all[:, t : t + 1], 1.0
                    )
                    nc.vector.reciprocal(w_stage[:, t, 0:1], den_all[:, t : t + 1])
                    nc.vector.tensor_scalar(
                        w_stage[:, t, 1:2],
                        w_stage[:, t, 0:1],
                        -1.0,
                        1.0,
                        op0=mybir.AluOpType.mult,
                        op1=mybir.AluOpType.add,
                    )

            ow = out_w.rearrange("(t p) k -> p t k", p=P)
            oi = out_i.rearrange("(t p) k -> p t k", p=P)
            nc.gpsimd.dma_start(ow, w_stage[:])
            nc.gpsimd.dma_start(oi, i_stage[:, :, 0:2])

    _split_multi_waits(nc, mybir)
    return nc


def _split_multi_waits(nc, mybir, max_waits=1):
    """This walrus build rejects instructions carrying more than one sync
    wait ("Too many sync wait commands"). Engines execute their queues in
    order, so an instruction with N waits is equivalent to N-1 preceding
    same-engine NoOps with one wait each plus the instruction keeping one."""
    for f in nc.m.functions:
        for bb in f.blocks:
            new = []
            for inst in bb.instructions:
                si = inst.sync_info
                waits = list(si.on_wait) if (si is not None and si.on_wait) else []
                if len(waits) > max_waits:
                    for k, w in enumerate(waits[:-max_waits]):
                        new.append(
                            mybir.InstNoOp(
                                name=f"{inst.name}-wsplit{k}",
                                engine=inst.engine,
                                ins=[],
                                outs=[],
                                sync_info=mybir.SyncInfo(
                                    on_wait=[w], on_update=[]
                                ),
                            )
                        )
                    inst.sync_info = mybir.SyncInfo(
                        on_wait=waits[-max_waits:],
                        on_update=list(si.on_update or []),
                    )
                new.append(inst)
            bb.instructions = new


def _run(x, W, trace=False):
    from concourse.bass_utils import run_bass_kernel_spmd

    if "nc" not in _cached:
        _cached["nc"] = _build_bass()
    nc = _cached["nc"]

    x = np.ascontiguousarray(np.asarray(x, dtype=np.float32))
    W = np.ascontiguousarray(np.asarray(W, dtype=np.float32))
    in_maps = [
        {
            "x": np.ascontiguousarray(x[c * T_CORE : (c + 1) * T_CORE]),
            "w": W,
        }
        for c in range(N_CORES)
    ]
    res = run_bass_kernel_spmd(
        nc,
        in_maps,
        core_ids=list(range(N_CORES)),
        trace=trace,
    )
    w_full = np.concatenate([r["out_w"] for r in res.results], axis=0)
    i_full = np.concatenate([r["out_i"] for r in res.results], axis=0)
    return (w_full, i_full.astype(np.int32)), res


def kernel(x, W):
    out, _ = _run(x, W, trace=False)
    return out


# revision 11
# speedup vs baseline: 1.4247x; 1.4247x over previous
"""MoE router gate kernel for Trainium2 (8 NeuronCores, SPMD data-parallel).

For x [16384, 2048] f32, W [8, 2048] f32:
    logits = x @ W.T; probs = softmax(logits); top-2; renormalize
returns (weights [T,2] f32, idx [T,2] int32).

Math: renormalized top-2 softmax weights depend only on the top-2 logits:
    w0 = 1/(1+exp(l1-l0)), w1 = 1 - w0.
Top-2 via the DVE Max8/MaxIndex8 instructions (top-8 sorted, take 2).

Per-core (2048 tokens) pipeline, all stages overlapped:
  1. DMA: natural-layout loads at full HBM rate (32KB contiguous per
     partition): S1[p, tq, d] = x[512g + 4p + tq, d], 4 MiB per group.
  2. DVE StreamTranspose (32x32 block transpose) swaps partition-low-5
     (token bits tl) with free-low-5 (d-low bits dl):
       S2[32r + dl, (tq, dblk, tl)] = x[512g + 128r + 4tl + tq, 32dblk + dl]
     giving contraction-ready d-on-partition layout without slow strided DMA.
  3. PE: 4-way ROW-TILED fp32 matmuls (tile r = token subset at partitions
     [32r, 32r+32), K=32): lhsT = W-slice [32, 8], rhs = S2 slice [32, 128],
     accumulating 64 d-chunks into psum [8, 128] per tile. Each row-tile
     yields complete logitsT for its own 128 tokens.
  4. PE transpose [8,128] -> [128, 8] token-major logits; DVE top-2 epilogue.

W is loaded naturally and rearranged on-chip (DVE blockT + replicas): the
direct strided W gather would cost ~64K 4-byte DMA descriptors (~45 us).
"""

import numpy as np

T_FULL = 16384
D = 2048
E = 8
N_CORES = 8
T_CORE = T_FULL // N_CORES  # 2048
P = 128
N_G = 4  # groups of 512 tokens
N_SLOT = 16  # (g, r) slots of 128 tokens

_cached = {}


def _build_bass():
    import concourse.bass as bass
    import concourse.mybir as mybir
    import concourse.tile as tile
    from concourse.masks import make_identity

    f32 = mybir.dt.float32
    u32 = mybir.dt.uint32

    nc = bass.Bass()
    x_in = nc.dram_tensor("x", [T_CORE, D], f32, kind="ExternalInput")
    w_in = nc.dram_tensor("w", [E, D], f32, kind="ExternalInput")
    out_w = nc.dram_tensor("out_w", [T_CORE, 2], f32, kind="ExternalOutput")
    out_i = nc.dram_tensor("out_i", [T_CORE, 2], u32, kind="ExternalOutput")

    # S1[p, tq, d] <- x[512g + 4p + tq, d] : 32KB contiguous per partition
    xv = x_in.rearrange("(g p tq) d -> g p tq d", g=N_G, tq=4, p=P)
    # W natural view for wtmp[4e + Bh, dq] = W[e, 512Bh + dq]
    wv = w_in.rearrange("e (Bh dq) -> e Bh dq", Bh=4)

    with tile.TileContext(nc) as tc:
        with (
            tc.tile_pool(name="const", bufs=1) as cpool,
            tc.tile_pool(name="s1", bufs=2) as s1pool,
            tc.tile_pool(name="s2", bufs=2) as s2pool,
            tc.tile_pool(name="small", bufs=2) as spool,
            tc.tile_pool(name="psum", bufs=1, space="PSUM") as ppool,
            tc.tile_pool(name="tpsum", bufs=2, space="PSUM") as tpool,
            tc.tile_pool(name="dummy_psum", bufs=1, space="PSUM") as dpool,
        ):
            dps = dpool.tile([1, 1], f32)

            def pe_gate(ap):
                nc.tensor.matmul(dps[:], ap, ap, start=True, stop=True)

            # ---- W preparation (tiny, off the critical path) ----
            # wtmp[(4e + Bh), (dmid*32 + dl)] = W[e, 512Bh + 32dmid + dl]
            wtmp = cpool.tile([32, 512], f32)
            for e in range(E):
                nc.sync.dma_start(wtmp[4 * e : 4 * e + 4, :], wv[e])
            # blockT -> wrep[dl, dmid*32 + (4e + Bh)] = same W element
            # free pos = dmid*32 + 4e + Bh -> dims [dmid, e, Bh]
            wrep = cpool.tile([P, 16, 8, 4], f32)
            nc.vector.transpose(
                wrep[0:32].rearrange("p a b c -> p (a b c)"), wtmp[:]
            )
            for r in range(1, 4):
                nc.sync.dma_start(
                    wrep[32 * r : 32 * (r + 1)].rearrange("p a b c -> p (a b c)"),
                    wrep[0:32].rearrange("p a b c -> p (a b c)"),
                )
            pe_gate(wrep[:, 0:1, 0, 0])

            ident8 = cpool.tile([E, E], f32)
            make_identity(nc, ident8[:])

            # persistent epilogue staging (disjoint columns per slot -> no
            # WAR/WAW deps; this walrus build allows only 1 wait/instruction)
            m8_all = cpool.tile([P, N_SLOT, 8], f32)
            i_stage = cpool.tile([P, N_SLOT, 8], u32)
            w_stage = cpool.tile([P, N_SLOT, 2], f32)
            neg_all = cpool.tile([P, N_SLOT], f32)
            ex_all = cpool.tile([P, N_SLOT], f32)
            den_all = cpool.tile([P, N_SLOT], f32)

            for g in range(N_G):
                s1 = s1pool.tile([P, 4, D], f32)
                nc.sync.dma_start(s1[:], xv[g])

                s2 = s2pool.tile([P, 4, 64, 32], f32)
                nc.vector.transpose(
                    s2[:].rearrange("p tq dblk tl -> p (tq dblk tl)"),
                    s1[:].rearrange("p tq d -> p (tq d)"),
                )

                pss = []
                for r in range(4):
                    psr = ppool.tile([E, P], f32, tag=f"ps{r}")
                    pss.append(psr)
                for j in range(64):
                    Bh, dmid = j // 16, j % 16
                    for r in range(4):
                        nc.tensor.matmul(
                            pss[r][:],
                            wrep[32 * r : 32 * (r + 1), dmid, :, Bh],
                            s2[32 * r : 32 * (r + 1), :, j, :],
                            start=(j == 0),
                            stop=(j == 63),
                            tile_position=(32 * r, 0),
                            skip_group_check=True,
                        )

                # pss[r][e, c=(tq, tl)] = logit of token 512g + 128r + 4tl
                # + tq. Reorder cols so transpose output partition u = 4tl +
                # tq = token offset in the r-block.
                lsb = spool.tile([E, 4, 32, 4], f32, tag="lsb")
                for r in range(4):
                    nc.scalar.copy(
                        lsb[:, r, :, :],
                        pss[r][:].rearrange("e (tq tl) -> e tl tq", tq=4),
                    )
                for r in range(4):
                    s = 4 * g + r
                    pt = tpool.tile([P, E], f32)
                    nc.tensor.transpose(pt[:], lsb[:, r, :, :], ident8[:])

                    m8 = m8_all[:, s, :]
                    nc.vector.max(m8, pt[:])
                    nc.vector.max_index(i_stage[:, s, :], m8, pt[:])
                    # w0 = 1/(1 + exp(l1 - l0)); w1 = 1 - w0
                    nc.gpsimd.tensor_scalar_mul(
                        neg_all[:, s : s + 1], m8[:, 0:1], -1.0
                    )
                    nc.scalar.activation(
                        ex_all[:, s : s + 1],
                        m8[:, 1:2],
                        mybir.ActivationFunctionType.Exp,
                        bias=neg_all[:, s : s + 1],
                    )
                    nc.gpsimd.tensor_scalar_add(
                        den_all[:, s : s + 1], ex_all[:, s : s + 1], 1.0
                    )
                    nc.vector.reciprocal(
                        w_stage[:, s, 0:1], den_all[:, s : s + 1]
                    )
                    nc.gpsimd.tensor_scalar(
                        w_stage[:, s, 1:2],
                        w_stage[:, s, 0:1],
                        -1.0,
                        1.0,
                        op0=mybir.AluOpType.mult,
                        op1=mybir.AluOpType.add,
                    )

            # token t = 128*slot + partition  ->  "(s p) k" view
            ow = out_w.rearrange("(s p) k -> p s k", p=P)
            oi = out_i.rearrange("(s p) k -> p s k", p=P)
            nc.gpsimd.dma_start(ow, w_stage[:])
            nc.gpsimd.dma_start(oi, i_stage[:, :, 0:2])

    _split_multi_waits(nc, mybir)
    return nc


def _split_multi_waits(nc, mybir, max_waits=1):
    """This walrus build rejects instructions carrying more than one sync
    wait. Engines execute their queues in order, so an instruction with N
    waits is equivalent to N-1 preceding same-engine NoOps with one wait
    each plus the instruction keeping one."""
    for f in nc.m.functions:
        for bb in f.blocks:
            new = []
            for inst in bb.instructions:
                si = inst.sync_info
                waits = list(si.on_wait) if (si is not None and si.on_wait) else []
                if len(waits) > max_waits:
                    for k, w in enumerate(waits[:-max_waits]):
                        new.append(
                            mybir.InstNoOp(
                                name=f"{inst.name}-wsplit{k}",
                                engine=inst.engine,
                                ins=[],
                                outs=[],
                                sync_info=mybir.SyncInfo(
                                    on_wait=[w], on_update=[]
                                ),
                            )
                        )
                    inst.sync_info = mybir.SyncInfo(
                        on_wait=waits[-max_waits:],
                        on_update=list(si.on_update or []),
                    )
                new.append(inst)
            bb.instructions = new


def _run(x, W, trace=False):
    from concourse.bass_utils import run_bass_kernel_spmd

    if "nc" not in _cached:
        _cached["nc"] = _build_bass()
    nc = _cached["nc"]

    x = np.ascontiguousarray(np.asarray(x, dtype=np.float32))
    W = np.ascontiguousarray(np.asarray(W, dtype=np.float32))
    in_maps = [
        {
            "x": np.ascontiguousarray(x[c * T_CORE : (c + 1) * T_CORE]),
            "w": W,
        }
        for c in range(N_CORES)
    ]
    res = run_bass_kernel_spmd(
        nc,
        in_maps,
        core_ids=list(range(N_CORES)),
        trace=trace,
    )
    w_full = np.concatenate([r["out_w"] for r in res.results], axis=0)
    i_full = np.concatenate([r["out_i"] for r in res.results], axis=0)
    return (w_full, i_full.astype(np.int32)), res


def kernel(x, W):
    out, _ = _run(x, W, trace=False)
    return out
